# revision 1
# baseline (speedup 1.0000x reference)
"""Multi-head cross attention on 8 Trainium2 NeuronCores.

Sharding: core c = b*4 + g handles batch b (of 2) and head-group g (4 heads
of the 16).  Each core projects Q/K/V for its 4 heads, runs attention, and
computes a partial output projection with its 256 rows of Wo; the host sums
the 4 partials per batch (plus bo and the bv@Wo term, exact because softmax
rows sum to 1).

Dataflow is fully "transposed" so no on-device transposes are needed:
  - host passes x^T in bf16 (transposed + cast on CPU)
  - QT[dh, sq]  = Wq_g.T @ xqT       (lhsT = Wq slice, rhs = xqT)
  - KT[dh, skv] = Wk_g.T @ xkvT      (two heads packed per 128-partition tile)
  - V[skv, dh]  = xkvT.T @ Wv'_g     (lhsT = xkvT slice, rhs = Wv' which has
                                      a zero column after each head, turned
                                      into ones -> fused softmax row-sums)
  - S^T[skv, q] = KT_h.T @ QT_h      (K=64; the two heads of a pair use
                                      disjoint PE row groups and execute
                                      concurrently; both land in one 1024-wide
                                      PSUM tile so one ACT Exp covers both)
  - P^T = exp(S^T / 8)               (no max subtraction; |scores/8| < ~3)
  - O'^T = [V_h|1|...].T @ P^T       (lhsT window is 128 wide so the PE array
                                      is fully used and the HAM clock gate
                                      stays at 2.4 GHz; rows 65-127 are junk,
                                      row 64 is the softmax row-sum)
  - O^T = O'^T[0:64] * (1/rowsum)    (reciprocal on DVE, broadcast across
                                      partitions on the idle GpSimd engine,
                                      multiply on DVE -- nothing touches the
                                      PE queue or PSUM, so block boundaries
                                      don't stall the matmul pipeline)
  - out_partial[sq, 1024] = O^T_allheads.T @ Wo_g
Attention runs in 8 blocks (head-pair x query-quarter); each block's
normalize is emitted two kv-tiles into the next block so its instructions
sit behind fresh matmul work in every engine queue.
Matmuls run in bf16 (fp32 PSUM accumulation; measured rel err ~4e-3).
fp32r (1 cyc/row in the cost model) measured ~3.5 cyc/row on HW and is
throttled in exactly the shapes attention needs, so bf16 wins.
"""

import sys

sys.path.insert(0, "/opt/trn_rl_repo")

import ml_dtypes
import numpy as np

BF16NP = ml_dtypes.bfloat16

B, SQ, SKV, D, H = 2, 2048, 2048, 1024, 16
DH = D // H          # 64
N_CORES = 8
G = 4                # head groups
HPG = H // G         # heads per group = 4
GC = HPG * DH        # group width = 256

_nc_cache = None


def _build_nc():
    import concourse.mybir as mybir
    import concourse.tile as tile
    from concourse import bacc

    F32 = mybir.dt.float32
    F32R = mybir.dt.float32r
    BF16 = mybir.dt.bfloat16
    AF = mybir.ActivationFunctionType
    MUL = mybir.AluOpType.mult

    nc = bacc.Bacc("TRN2", target_bir_lowering=False, debug=False,
                   num_devices=N_CORES)

    xqT_d = nc.dram_tensor("xqT", [D, SQ], BF16, kind="ExternalInput").ap()
    xkvT_d = nc.dram_tensor("xkvT", [D, SKV], BF16, kind="ExternalInput").ap()
    wq_d = nc.dram_tensor("wq", [D, GC], BF16, kind="ExternalInput").ap()
    wk_d = nc.dram_tensor("wk", [D, GC], BF16, kind="ExternalInput").ap()
    # Wv' with a zero column after each head's 64 (slots for the ones column)
    wvp_d = nc.dram_tensor("wvp", [D, HPG * 65], BF16, kind="ExternalInput").ap()
    wo_d = nc.dram_tensor("wo", [GC, D], BF16, kind="ExternalInput").ap()
    bq_d = nc.dram_tensor("bq2", [128, 2], F32, kind="ExternalInput").ap()
    bk_d = nc.dram_tensor("bk2", [128, 2], F32, kind="ExternalInput").ap()
    ones_d = nc.dram_tensor("ones64", [1, 128], F32R, kind="ExternalInput").ap()
    out_d = nc.dram_tensor("out_p", [SQ, D], F32, kind="ExternalOutput").ap()

    ND = D // 128        # 8 d-tiles (contraction over D)
    NJ = SKV // 128      # 16 kv tiles
    VW = HPG * 65        # 260, V' row width
    scale = 1.0 / float(np.sqrt(DH))

    with tile.TileContext(nc) as tc:
        with (
            tc.tile_pool(name="persist", bufs=1) as pp,
            tc.tile_pool(name="pha", bufs=1) as pa,
            tc.tile_pool(name="phb", bufs=1) as pb,
        ):
            # ---- persistent tiles -------------------------------------
            qt_sb = pp.tile([128, 2 * SQ], BF16, tag="qt_sb")
            kt_sb = pp.tile([128, 2 * SKV], BF16, tag="kt_sb")
            vp_sb = pp.tile([128, NJ * VW + 63], BF16, tag="vp_sb")
            o_sbA = pp.tile([128, 2 * 1024], BF16, tag="o_sbA")
            o_sbB = pp.tile([128, 2 * 1024], BF16, tag="o_sbB")
            bq_sb = pp.tile([128, 2], F32, tag="bq_sb")
            bk_sb = pp.tile([128, 2], F32, tag="bk_sb")
            ones_sb = pp.tile([1, 128], F32R, tag="ones_sb")
            nc.sync.dma_start(out=bq_sb[:], in_=bq_d[:])
            nc.sync.dma_start(out=bk_sb[:], in_=bk_d[:])
            nc.sync.dma_start(out=ones_sb[:], in_=ones_d[:])

            # ---- phase A: load xkvT, weights; compute KT and V' -------
            wk_sb = pa.tile([128, ND * GC], BF16, tag="wk_sb")
            wvp_sb = pa.tile([128, ND * VW], BF16, tag="wvp_sb")
            for d in range(ND):
                nc.sync.dma_start(
                    out=wk_sb[:, d * GC:(d + 1) * GC],
                    in_=wk_d[d * 128:(d + 1) * 128, :])
            xkv = []
            for d in range(ND):
                t = pa.tile([128, SKV], BF16, tag=f"xkv{d}", name=f"xkv{d}")
                nc.gpsimd.dma_start(out=t[:], in_=xkvT_d[d * 128:(d + 1) * 128, :])
                xkv.append(t)

            with tc.tile_pool(name="psA", bufs=1, space="PSUM") as psA:
                # KT (2 pair-tiles x 4 q chunks); d-outer so each xkv DMA
                # tile is consumed as soon as it lands
                pk = {}
                for p in range(2):
                    for qc in range(4):
                        pk[p, qc] = psA.tile([128, 512], F32, tag="pk",
                                             bufs=8, name=f"pk{p}{qc}")
                for d in range(ND):
                    for p in range(2):
                        for qc in range(4):
                            nc.tensor.matmul(
                                pk[p, qc][:],
                                wk_sb[:, d * GC + p * 128:d * GC + (p + 1) * 128],
                                xkv[d][:, qc * 512:(qc + 1) * 512],
                                start=(d == 0), stop=(d == ND - 1),
                            )
                for p in range(2):
                    for qc in range(4):
                        nc.scalar.activation(
                            kt_sb[:, p * SKV + qc * 512:p * SKV + (qc + 1) * 512],
                            pk[p, qc][:], AF.Identity, bias=bk_sb[:, p:p + 1])
                # V' (16 kv tiles, accumulate over d)
                for d in range(ND):
                    nc.sync.dma_start(
                        out=wvp_sb[:, d * VW:(d + 1) * VW],
                        in_=wvp_d[d * 128:(d + 1) * 128, :])
                for j in range(NJ):
                    pv = psA.tile([128, VW], F32, tag="pk", bufs=8,
                                  name=f"pv{j}")
                    for d in range(ND):
                        nc.tensor.matmul(
                            pv[:],
                            xkv[d][:, j * 128:(j + 1) * 128],
                            wvp_sb[:, d * VW:(d + 1) * VW],
                            start=(d == 0), stop=(d == ND - 1),
                        )
                    nc.vector.tensor_copy(vp_sb[:, j * VW:(j + 1) * VW], pv[:])
                # ones columns of V' (stride-65 view hits col 64 of each head)
                oc = vp_sb[:, 64:NJ * VW:65]
                nc.scalar.activation(oc, oc, AF.Copy, scale=0.0, bias=1.0)
                # zero tail pad (scale-0 copy from finite psum keeps NaNs out)
                nc.scalar.activation(vp_sb[:, NJ * VW:NJ * VW + 63],
                                     pv[:, 0:63], AF.Copy, scale=0.0)

                # ---- phase B: stream xqT, compute QT ------------------
                wq_sb = pb.tile([128, ND * GC], BF16, tag="wq_sb")
                for d in range(ND):
                    nc.sync.dma_start(
                        out=wq_sb[:, d * GC:(d + 1) * GC],
                        in_=wq_d[d * 128:(d + 1) * 128, :])
                xq_tiles = []
                for d in range(ND):
                    xq_t = pb.tile([128, SQ], BF16, tag="xq", bufs=3,
                                   name=f"xq{d}")
                    nc.gpsimd.dma_start(out=xq_t[:],
                                        in_=xqT_d[d * 128:(d + 1) * 128, :])
                    xq_tiles.append(xq_t)
                pq = {}
                for p in range(2):
                    for qc in range(4):
                        pq[p, qc] = psA.tile([128, 512], F32, tag="pk", bufs=8,
                                             name=f"pq{p}{qc}")
                for d in range(ND):
                    xq_t = xq_tiles[d]
                    for p in range(2):
                        for qc in range(4):
                            nc.tensor.matmul(
                                pq[p, qc][:],
                                wq_sb[:, d * GC + p * 128:d * GC + (p + 1) * 128],
                                xq_t[:, qc * 512:(qc + 1) * 512],
                                start=(d == 0), stop=(d == ND - 1),
                            )
                for p in range(2):
                    for qc in range(4):
                        blk = slice(p * SQ + qc * 512, p * SQ + (qc + 1) * 512)
                        nc.scalar.activation(
                            qt_sb[:, blk], pq[p, qc][:],
                            AF.Identity, bias=bq_sb[:, p:p + 1])

            # ---- attention -------------------------------------------
            with (
                tc.tile_pool(name="attn", bufs=1) as at,
                tc.tile_pool(name="psC", bufs=1, space="PSUM") as psC,
                tc.tile_pool(name="oproj", bufs=1) as op_pool,
                tc.tile_pool(name="psD", bufs=1, space="PSUM") as psD,
            ):
                wo_sb = op_pool.tile([128, 2 * D], BF16, tag="wo_sb")
                nc.sync.dma_start(
                    out=wo_sb[:].rearrange("p (t n) -> p t n", t=2),
                    in_=wo_d.rearrange("(t p) n -> p t n", p=128),
                )

                def emit_outproj(lo, hi):
                    for s in range(lo, hi):
                        for n2 in range(2):
                            po = psD.tile([128, 512], F32, tag="po", bufs=2,
                                          name=f"po{s}{n2}")
                            o_half = o_sbA if s < 8 else o_sbB
                            s8 = s % 8
                            for tt in range(2):
                                nc.tensor.matmul(
                                    po[:],
                                    o_half[:, tt * 1024 + s8 * 128:
                                           tt * 1024 + (s8 + 1) * 128],
                                    wo_sb[:, tt * D + n2 * 512:
                                          tt * D + n2 * 512 + 512],
                                    start=(tt == 0), stop=(tt == 1),
                                )
                            ob = op_pool.tile([128, 512], F32, tag="ob",
                                              bufs=3, name=f"ob{s}{n2}")
                            nc.vector.tensor_copy(ob[:], po[:])
                            nc.sync.dma_start(
                                out=out_d[s * 128:(s + 1) * 128,
                                          n2 * 512:(n2 + 1) * 512],
                                in_=ob[:])

                pending_norm = []

                def flush_norm():
                    while pending_norm:
                        pending_norm.pop(0)()

                for t in range(2):          # head pair
                    for qq in range(4):     # q quarter (512)
                        o_ps = {}
                        for hp in range(2):
                            o_ps[hp] = psC.tile(
                                [128, 512], F32, tag="o_ps", bufs=2,
                                name=f"o_ps{t}{qq}{hp}")
                        for j in range(NJ):
                            st = psC.tile([128, 1024], F32, tag="st2", bufs=2,
                                          name=f"st{t}{qq}{j}")
                            # K=64 scores; the two heads use disjoint row
                            # groups (partitions 0-63 / 64-127) and execute
                            # concurrently on the PE
                            for hp in range(2):
                                nc.tensor.matmul(
                                    st[:, hp * 512:(hp + 1) * 512],
                                    kt_sb[hp * 64:(hp + 1) * 64,
                                          t * SKV + j * 128:
                                          t * SKV + (j + 1) * 128],
                                    qt_sb[hp * 64:(hp + 1) * 64,
                                          t * SQ + qq * 512:
                                          t * SQ + (qq + 1) * 512],
                                    start=True, stop=True,
                                )
                            p_t = at.tile([128, 1024], BF16, tag="pt",
                                          bufs=6, name=f"pt{t}{qq}{j}")
                            nc.scalar.activation(p_t[:], st[:],
                                                 AF.Exp, scale=scale)
                            for hp in range(2):
                                h = 2 * t + hp
                                nc.tensor.matmul(
                                    o_ps[hp][:],
                                    vp_sb[:, j * VW + h * 65:
                                          j * VW + h * 65 + 128],
                                    p_t[:, hp * 512:(hp + 1) * 512],
                                    start=(j == 0), stop=(j == NJ - 1),
                                )
                            if j == 1 and t == 1 and qq == 3:
                                # o_sbA's last normalize (t1,qq1) is already
                                # emitted; its outproj half can gap-fill the
                                # PE during the final attention blocks
                                flush_norm()
                                emit_outproj(0, 8)
                            elif j == 1:
                                # emit the previous block's normalize now --
                                # its bc matmuls land behind this block's
                                # first scores in the PE stream, so the PE
                                # never head-of-line blocks on the slow
                                # reciprocal chain
                                flush_norm()
                        # stage rowsums out of PSUM quickly, then queue the
                        # rest of the normalize for later emission
                        for hp in range(2):
                            ot = at.tile([64, 512], F32, tag="ot",
                                         bufs=4, name=f"ot{t}{qq}{hp}")
                            nc.vector.tensor_copy(ot[:], o_ps[hp][0:64, :])
                            rs = at.tile([1, 512], F32, tag="rs", bufs=4,
                                         name=f"rs{t}{qq}{hp}")
                            nc.vector.tensor_copy(rs[:], o_ps[hp][64:65, :])

                            def norm(t=t, qq=qq, hp=hp, ot=ot, rs=rs):
                                rcp = at.tile([1, 512], F32, tag="rcp",
                                              bufs=4, name=f"rcp{t}{qq}{hp}")
                                nc.vector.reciprocal(rcp[:], rs[:])
                                bcs = at.tile([64, 512], F32, tag="bcs",
                                              bufs=4, name=f"bcs{t}{qq}{hp}")
                                nc.gpsimd.partition_broadcast(
                                    bcs[:], rcp[:], channels=64)
                                o_half = o_sbA if qq < 2 else o_sbB
                                col = t * 1024 + (qq % 2) * 512
                                nc.vector.tensor_tensor(
                                    out=o_half[hp * 64:(hp + 1) * 64,
                                               col:col + 512],
                                    in0=ot[:], in1=bcs[:],
                                    op=MUL)

                            pending_norm.append(norm)
                flush_norm()

                # ---- output projection (second half; first half was
                # emitted inside the attention loop) ------------------------
                emit_outproj(8, 16)

    nc.compile()
    return nc


def build_in_maps(inputs):
    query_input = np.asarray(inputs["query_input"], dtype=np.float32)
    kv_input = np.asarray(inputs["kv_input"], dtype=np.float32)
    Wq = np.asarray(inputs["Wq"], dtype=np.float32)
    bq = np.asarray(inputs["bq"], dtype=np.float32)
    Wkv = np.asarray(inputs["Wkv"], dtype=np.float32)
    bkv = np.asarray(inputs["bkv"], dtype=np.float32)
    Wo = np.asarray(inputs["Wo"], dtype=np.float32)

    Wk = Wkv[:, :D]
    Wv = Wkv[:, D:]
    bk = bkv[:D]
    ones64 = np.ones((1, 128), np.float32)

    xT = [np.ascontiguousarray(query_input[b].T).astype(BF16NP) for b in range(B)]
    kvT = [np.ascontiguousarray(kv_input[b].T).astype(BF16NP) for b in range(B)]

    in_maps = []
    for c in range(N_CORES):
        b, g = divmod(c, G)
        c0 = g * GC
        wvp = np.zeros((D, HPG * 65), np.float32)
        for h in range(HPG):
                wvp[:, h * 65:h * 65 + 64] = Wv[:, c0 + h * DH:c0 + (h + 1) * DH]
        bq2 = bq[c0:c0 + GC].reshape(2, 128).T.copy()
        bk2 = bk[c0:c0 + GC].reshape(2, 128).T.copy()
        in_maps.append({
                "xqT": xT[b],
                "xkvT": kvT[b],
                "wq": np.ascontiguousarray(Wq[:, c0:c0 + GC]).astype(BF16NP),
                "wk": np.ascontiguousarray(Wk[:, c0:c0 + GC]).astype(BF16NP),
                "wvp": wvp.astype(BF16NP),
                "wo": np.ascontiguousarray(Wo[c0:c0 + GC, :]).astype(BF16NP),
                "bq2": np.ascontiguousarray(bq2),
                "bk2": np.ascontiguousarray(bk2),
                "ones64": ones64,
        })
    return in_maps


def kernel(query_input, kv_input, Wq, bq, Wkv, bkv, Wo, bo):
    global _nc_cache
    from concourse import bass_utils

    if _nc_cache is None:
        _nc_cache = _build_nc()
    nc = _nc_cache

    Wkv = np.asarray(Wkv, dtype=np.float32)
    Wo = np.asarray(Wo, dtype=np.float32)
    bo = np.asarray(bo, dtype=np.float32)
    bv = np.asarray(bkv, np.float32)[D:]

    in_maps = build_in_maps(dict(
        query_input=query_input, kv_input=kv_input, Wq=Wq, bq=bq,
        Wkv=Wkv, bkv=bkv, Wo=Wo))

    res = bass_utils.run_bass_kernel_spmd(nc, in_maps,
                                          core_ids=list(range(N_CORES)))

    # gather: sum the 4 head-group partials per batch; add biases the device
    # left out (bo, and bv which passes through Wo since softmax rows sum to 1)
    tail = bv @ Wo + bo
    out = np.empty((B, SQ, D), np.float32)
    for b in range(B):
        acc = res.results[b * G + 0]["out_p"].astype(np.float32).copy()
        for g in range(1, G):
                acc += res.results[b * G + g]["out_p"]
        out[b] = acc + tail[None, :]
    return out



# revision 11
# speedup vs baseline: 1.1836x; 1.1836x over previous
"""Multi-head cross attention on 8 Trainium2 NeuronCores.

Sharding: core c = b*4 + g handles batch b (of 2) and head-group g (4 heads
of the 16).  Each core projects Q/K/V for its 4 heads, runs attention, and
computes a partial output projection with its 256 rows of Wo; the host sums
the 4 partials per batch (plus bo and the bv@Wo term, exact because softmax
rows sum to 1).

Dataflow is fully "transposed" so no on-device transposes are needed:
  - host passes x^T in bf16 (transposed + cast on CPU)
  - QT[dh, sq]  = Wq_g.T @ xqT       (lhsT = Wq slice, rhs = xqT)
  - KT[dh, skv] = Wk_g.T @ xkvT      (two heads packed per 128-partition tile)
  - V[skv, dh]  = xkvT.T @ Wv'_g     (Wv' has a spare column after each head
                                      that is memset to ones -> AV's matmul
                                      also produces the softmax row-sums)
  - S^T[skv, q] = KT_h.T @ QT_h      (K=64; the two heads of a pair use
                                      disjoint PE row groups and execute
                                      concurrently; both land in one 1024-wide
                                      PSUM tile so one ACT Exp covers both)
  - P^T = exp(S^T / 8)               (no max subtraction; |scores/8| < ~3)
  - O'^T = [V_h|1|...].T @ P^T       (128-wide lhsT window, rows 65-127 junk,
                                      row 64 is the softmax row-sum)
  - O^T = O'^T[0:64] * (1/rowsum)    (reciprocal_approx_fast on DVE, gpsimd
                                      partition broadcast, DVE multiply)
  - out_partial[sq, 1024] = O^T_allheads.T @ Wo_g   (bf16 back to host)

Schedule: the scalar (ACT) engine's Exp stream is the hard floor (~128us of
exp work per core), so attention starts as soon as head-pair 0's K/Q
projections and the first two V tiles exist (~10us in).  Everything else --
remaining V tiles, head-pair-1 K/Q projections, the output projection -- is
emitted as small "trickle" quanta between attention iterations, filling the
PE/DVE slack under the Exp stream.  Mid-attention projection staging uses
DVE scalar_tensor_tensor (bias add + cast) so ACT is never interrupted.
Inputs stream in on three DMA queues (sync/scalar/gpsimd); the bf16 partial
output streams out per 128-row tile as soon as its normalize completes.
"""

import sys

sys.path.insert(0, "/opt/trn_rl_repo")

import ml_dtypes
import numpy as np

BF16NP = ml_dtypes.bfloat16

B, SQ, SKV, D, H = 2, 2048, 2048, 1024, 16
DH = D // H          # 64
N_CORES = 8
G = 4                # head groups
HPG = H // G         # heads per group = 4
GC = HPG * DH        # group width = 256

ND = D // 128        # 8 d-tiles (contraction over D)
NJ = SKV // 128      # 16 kv tiles
VW = HPG * 65        # 260, V' row width

_nc_cache = None


def _build_nc():
    import concourse.mybir as mybir
    import concourse.tile as tile
    from concourse import bacc

    F32 = mybir.dt.float32
    BF16 = mybir.dt.bfloat16
    AF = mybir.ActivationFunctionType
    ADD = mybir.AluOpType.add
    MUL = mybir.AluOpType.mult
    BYP = mybir.AluOpType.bypass

    nc = bacc.Bacc("TRN2", target_bir_lowering=False, debug=False,
                   num_devices=N_CORES)

    xqT_d = nc.dram_tensor("xqT", [D, SQ], BF16, kind="ExternalInput").ap()
    xkvT_d = nc.dram_tensor("xkvT", [D, SKV], BF16, kind="ExternalInput").ap()
    wq_d = nc.dram_tensor("wq", [D, GC], BF16, kind="ExternalInput").ap()
    wk_d = nc.dram_tensor("wk", [D, GC], BF16, kind="ExternalInput").ap()
    # Wv' with a zero column after each head's 64 (slots for the ones column)
    wvp_d = nc.dram_tensor("wvp", [D, VW], BF16, kind="ExternalInput").ap()
    wo_d = nc.dram_tensor("wo", [GC, D], BF16, kind="ExternalInput").ap()
    bq_d = nc.dram_tensor("bq2", [128, 2], F32, kind="ExternalInput").ap()
    bk_d = nc.dram_tensor("bk2", [128, 2], F32, kind="ExternalInput").ap()
    out_d = nc.dram_tensor("out_p", [SQ, D], BF16, kind="ExternalOutput").ap()

    scale = 1.0 / float(np.sqrt(DH))

    with tile.TileContext(nc) as tc:
        with (
            tc.tile_pool(name="persist", bufs=1) as pp,
            tc.tile_pool(name="attn", bufs=1) as at,
            tc.tile_pool(name="ps", bufs=1, space="PSUM") as ps,
        ):
            # ---- persistent SBUF ---------------------------------------
            qt_sb = pp.tile([128, 2 * SQ], BF16, tag="qt_sb")
            kt_sb = pp.tile([128, 2 * SKV], BF16, tag="kt_sb")
            vp_sb = pp.tile([128, NJ * VW + 63], BF16, tag="vp_sb")
            o_sbA = pp.tile([128, 2 * 1024], BF16, tag="o_sbA")
            o_sbB = pp.tile([128, 2 * 1024], BF16, tag="o_sbB")
            wq_sb = pp.tile([128, ND * GC], BF16, tag="wq_sb")
            wk_sb = pp.tile([128, ND * GC], BF16, tag="wk_sb")
            wvp_sb = pp.tile([128, ND * VW], BF16, tag="wvp_sb")
            wo_sb = pp.tile([128, 2 * D], BF16, tag="wo_sb")
            bq_sb = pp.tile([128, 2], F32, tag="bq_sb")
            bk_sb = pp.tile([128, 2], F32, tag="bk_sb")
            xkv = [pp.tile([128, SKV], BF16, tag=f"xkv{d}", name=f"xkv{d}")
                   for d in range(ND)]
            xq = [pp.tile([128, SQ], BF16, tag=f"xq{d}", name=f"xq{d}")
                  for d in range(ND)]

            # ---- input DMAs on three queues ----------------------------
            # sync queue: K/Q weights first, then xq quarter-major so the
            # qq0 columns land before attention needs them.
            nc.sync.dma_start(
                out=wk_sb[:].rearrange("p (t n) -> p t n", t=ND),
                in_=wk_d.rearrange("(t p) n -> p t n", p=128))
            nc.sync.dma_start(
                out=wq_sb[:].rearrange("p (t n) -> p t n", t=ND),
                in_=wq_d.rearrange("(t p) n -> p t n", p=128))
            for qq in range(4):
                for d in range(ND):
                    nc.sync.dma_start(
                        out=xq[d][:, qq * 512:(qq + 1) * 512],
                        in_=xqT_d[d * 128:(d + 1) * 128,
                                  qq * 512:(qq + 1) * 512])
            # scalar queue (idle until the first exp): odd xkv tiles, V'
            # weights, biases.
            nc.scalar.dma_start(out=bq_sb[:], in_=bq_d[:])
            nc.scalar.dma_start(out=bk_sb[:], in_=bk_d[:])
            for d in (1, 3, 5, 7):
                nc.scalar.dma_start(out=xkv[d][:],
                                    in_=xkvT_d[d * 128:(d + 1) * 128, :])
            nc.scalar.dma_start(
                out=wvp_sb[:].rearrange("p (t n) -> p t n", t=ND),
                in_=wvp_d.rearrange("(t p) n -> p t n", p=128))
            # gpsimd queue: even xkv tiles, Wo.
            for d in (0, 2, 4, 6):
                nc.gpsimd.dma_start(out=xkv[d][:],
                                    in_=xkvT_d[d * 128:(d + 1) * 128, :])
            nc.gpsimd.dma_start(
                out=wo_sb[:].rearrange("p (t n) -> p t n", t=2),
                in_=wo_d.rearrange("(t p) n -> p t n", p=128))
            # zero the vp tail pad once (j=15 h=3 AV window reads into it)
            nc.gpsimd.memset(vp_sb[:, NJ * VW:NJ * VW + 63], 0.0)
            # ones columns of V' (stride-65 hits col 0 of each head), set
            # once at startup; the per-tile stage copies never touch them
            nc.gpsimd.memset(vp_sb[:, 64:NJ * VW:65], 1.0)

            # ---- helpers ----------------------------------------------
            def emit_kt(p, use_act):
                """K^T for head pair p: 2 score-sized PSUM tiles hold the 4
                kv chunks, d-outer so DMA-landed xkv tiles are consumed
                immediately (pre-attention) -- for the trickled p=1 the
                tiles are all resident anyway."""
                kst = [ps.tile([128, 1024], F32, tag="st", bufs=2,
                               name=f"kst{p}{i}") for i in range(2)]
                for di, d in enumerate((0, 1, 2, 3, 4, 5, 6, 7)):
                    for qc in range(4):
                        nc.tensor.matmul(
                            kst[qc // 2][:, (qc % 2) * 512:(qc % 2 + 1) * 512],
                            wk_sb[:, d * GC + p * 128:d * GC + (p + 1) * 128],
                            xkv[d][:, qc * 512:(qc + 1) * 512],
                            start=(di == 0), stop=(di == ND - 1))
                for qc in range(4):
                    dst = kt_sb[:, p * SKV + qc * 512:p * SKV + (qc + 1) * 512]
                    src = kst[qc // 2][:, (qc % 2) * 512:(qc % 2 + 1) * 512]
                    nc.scalar.activation(dst, src, AF.Identity,
                                         bias=bk_sb[:, p:p + 1])

            def emit_kt_qc(p, qc):
                """One kv chunk of K^T for a trickled pair (d-inner, one
                small PSUM tile, DVE staging)."""
                kp = ps.tile([128, 512], F32, tag="gp", bufs=2,
                             name=f"kp{p}{qc}")
                for d in range(ND):
                    nc.tensor.matmul(
                        kp[:],
                        wk_sb[:, d * GC + p * 128:d * GC + (p + 1) * 128],
                        xkv[d][:, qc * 512:(qc + 1) * 512],
                        start=(d == 0), stop=(d == ND - 1))
                dst = kt_sb[:, p * SKV + qc * 512:p * SKV + (qc + 1) * 512]
                nc.scalar.activation(dst, kp[:], AF.Identity,
                                     bias=bk_sb[:, p:p + 1])

            def emit_qt(p, qq, use_act):
                qp = ps.tile([128, 512], F32, tag="gp", bufs=2,
                             name=f"qp{p}{qq}")
                for d in range(ND):
                    nc.tensor.matmul(
                        qp[:],
                        wq_sb[:, d * GC + p * 128:d * GC + (p + 1) * 128],
                        xq[d][:, qq * 512:(qq + 1) * 512],
                        start=(d == 0), stop=(d == ND - 1))
                dst = qt_sb[:, p * SQ + qq * 512:p * SQ + (qq + 1) * 512]
                if use_act:
                    nc.scalar.activation(dst, qp[:], AF.Identity,
                                         bias=bq_sb[:, p:p + 1])
                else:
                    nc.scalar.activation(dst, qp[:], AF.Identity,
                                         bias=bq_sb[:, p:p + 1])

            def emit_vp(j):
                """V' kv-tile j for all 4 heads (+ ones columns)."""
                pv = ps.tile([128, VW], F32, tag="gp", bufs=2, name=f"pv{j}")
                for d in range(ND):
                    nc.tensor.matmul(
                        pv[:],
                        xkv[d][:, j * 128:(j + 1) * 128],
                        wvp_sb[:, d * VW:(d + 1) * VW],
                        start=(d == 0), stop=(d == ND - 1))
                # strided copy skips each head's ones column (col 64 of
                # 65), which a single startup memset owns race-free
                dst3 = vp_sb[:, j * VW:(j + 1) * VW].rearrange(
                    "p (h c) -> p h c", h=HPG)
                src3 = pv[:].rearrange("p (h c) -> p h c", h=HPG)
                nc.vector.tensor_copy(dst3[:, :, 0:64], src3[:, :, 0:64])

            def emit_outproj(s, n2):
                po = ps.tile([128, 512], F32, tag="gp", bufs=2,
                             name=f"po{s}{n2}")
                o_half = o_sbA if s < 8 else o_sbB
                s8 = s % 8
                for tt in range(2):
                    nc.tensor.matmul(
                        po[:],
                        o_half[:, tt * 1024 + s8 * 128:
                               tt * 1024 + (s8 + 1) * 128],
                        wo_sb[:, tt * D + n2 * 512:tt * D + n2 * 512 + 512],
                        start=(tt == 0), stop=(tt == 1))
                ob = at.tile([128, 512], BF16, tag="ob", bufs=4,
                             name=f"ob{s}{n2}")
                nc.vector.tensor_copy(ob[:], po[:])
                nc.sync.dma_start(
                    out=out_d[s * 128:(s + 1) * 128,
                              n2 * 512:(n2 + 1) * 512],
                    in_=ob[:])

            # ---- pre-attention: pair-0 K/Q, first V' tiles -------------
            emit_kt(0, use_act=True)
            emit_qt(0, 0, use_act=True)
            emit_vp(0)
            emit_vp(1)

            # trickle quanta, popped one per attention iteration
            # slot i is popped in attention iteration i; vp[j] is consumed in
            # iteration j (t0,qq0), qt(0,qq) in iteration 16*qq, pair-1 K/Q
            # from iteration 64 on.
            trickles = []
            for j in range(2, NJ):
                trickles.append(lambda j=j: emit_vp(j))           # slots 0-13
            trickles.append(lambda: emit_qt(0, 1, use_act=False))  # slot 14
            trickles.append(lambda: emit_qt(0, 2, use_act=False))
            trickles.append(lambda: emit_qt(0, 3, use_act=False))
            for qc in range(4):
                trickles.append(lambda qc=qc: emit_kt_qc(1, qc))
            for qq in range(4):
                trickles.append(lambda qq=qq: emit_qt(1, qq, use_act=False))

            pending_norm = []

            def flush_norm():
                while pending_norm:
                    pending_norm.pop(0)()

            # ---- attention --------------------------------------------
            for t in range(2):          # head pair
                for qq in range(4):     # q quarter (512)
                    o_ps = {}
                    for hp in range(2):
                        o_ps[hp] = ps.tile([128, 512], F32, tag="o",
                                           bufs=2, name=f"o{t}{qq}{hp}")
                    for j in range(NJ):
                        st = ps.tile([128, 1024], F32, tag="st", bufs=2,
                                     name=f"st{t}{qq}{j}")
                        # K=64 scores; the two heads use disjoint row groups
                        # (partitions 0-63 / 64-127), executing concurrently
                        for hp in range(2):
                            nc.tensor.matmul(
                                st[:, hp * 512:(hp + 1) * 512],
                                kt_sb[hp * 64:(hp + 1) * 64,
                                      t * SKV + j * 128:
                                      t * SKV + (j + 1) * 128],
                                qt_sb[hp * 64:(hp + 1) * 64,
                                      t * SQ + qq * 512:
                                      t * SQ + (qq + 1) * 512],
                                start=True, stop=True)
                        p_t = at.tile([128, 1024], BF16, tag="pt",
                                      bufs=6, name=f"pt{t}{qq}{j}")
                        nc.scalar.activation(p_t[:], st[:], AF.Exp,
                                             scale=scale)
                        for hp in range(2):
                            h = 2 * t + hp
                            nc.tensor.matmul(
                                o_ps[hp][:],
                                vp_sb[:, j * VW + h * 65:
                                      j * VW + h * 65 + 128],
                                p_t[:, hp * 512:(hp + 1) * 512],
                                start=(j == 0), stop=(j == NJ - 1))
                        if j == 1:
                            # emit the previous block's normalize now -- its
                            # DVE/gpsimd ops land behind two fresh iterations
                            # of work in every engine queue
                            flush_norm()
                        if trickles and ((t == 0 and qq == 0)
                                         or j % 2 == 1):
                            trickles.pop(0)()
                    # quick-drain the accumulators (frees the PSUM ring for
                    # the next block); the slow normalize chain is deferred
                    ovs = {}
                    for hp in range(2):
                        ov = at.tile([64, 512], F32, tag="ov", bufs=4,
                                     name=f"ov{t}{qq}{hp}")
                        nc.vector.tensor_copy(ov[:], o_ps[hp][0:64, :])
                        rs = at.tile([1, 512], F32, tag="rs", bufs=4,
                                     name=f"rs{t}{qq}{hp}")
                        nc.vector.tensor_copy(rs[:], o_ps[hp][64:65, :])
                        ovs[hp] = (ov, rs)

                    def norm(t=t, qq=qq, ovs=ovs):
                        for hp in range(2):
                            ov, rs = ovs[hp]
                            # broadcast the raw rowsum (from the relocated
                            # partition-0 tile -- gpsimd broadcast and the
                            # custom-DVE reciprocal both mangle
                            # partition-offset inputs on HW), then
                            # reciprocal on the partition-0 broadcast
                            bcs = at.tile([64, 512], F32, tag="bcs", bufs=4,
                                          name=f"bcs{t}{qq}{hp}")
                            nc.gpsimd.partition_broadcast(
                                bcs[:], rs[:], channels=64)
                            rcb = at.tile([64, 512], F32, tag="rcb", bufs=4,
                                          name=f"rcb{t}{qq}{hp}")
                            nc.vector.reciprocal_approx_fast(rcb[:], bcs[:])
                            o_half = o_sbA if qq < 2 else o_sbB
                            col = t * 1024 + (qq % 2) * 512
                            nc.vector.tensor_tensor(
                                out=o_half[hp * 64:(hp + 1) * 64,
                                           col:col + 512],
                                in0=ov[:], in1=rcb[:], op=MUL)
                        if t == 1:
                            # this q-quarter's rows are complete: stream its
                            # output projection through the trickle queue
                            for k in range(4):
                                s = qq * 4 + k
                                for n2 in range(2):
                                    trickles.append(
                                        lambda s=s, n2=n2: emit_outproj(s, n2))

                    pending_norm.append(norm)
            flush_norm()
            while trickles:
                trickles.pop(0)()

    nc.compile()
    return nc


def build_in_maps(inputs):
    query_input = np.asarray(inputs["query_input"], dtype=np.float32)
    kv_input = np.asarray(inputs["kv_input"], dtype=np.float32)
    Wq = np.asarray(inputs["Wq"], dtype=np.float32)
    bq = np.asarray(inputs["bq"], dtype=np.float32)
    Wkv = np.asarray(inputs["Wkv"], dtype=np.float32)
    bkv = np.asarray(inputs["bkv"], dtype=np.float32)
    Wo = np.asarray(inputs["Wo"], dtype=np.float32)

    Wk = Wkv[:, :D]
    Wv = Wkv[:, D:]
    bk = bkv[:D]

    xT = [np.ascontiguousarray(query_input[b].T).astype(BF16NP) for b in range(B)]
    kvT = [np.ascontiguousarray(kv_input[b].T).astype(BF16NP) for b in range(B)]

    in_maps = []
    for c in range(N_CORES):
        b, g = divmod(c, G)
        c0 = g * GC
        wvp = np.zeros((D, HPG * 65), np.float32)
        for h in range(HPG):
                wvp[:, h * 65:h * 65 + 64] = Wv[:, c0 + h * DH:c0 + (h + 1) * DH]
        bq2 = bq[c0:c0 + GC].reshape(2, 128).T.copy()
        bk2 = bk[c0:c0 + GC].reshape(2, 128).T.copy()
        in_maps.append({
                "xqT": xT[b],
                "xkvT": kvT[b],
                "wq": np.ascontiguousarray(Wq[:, c0:c0 + GC]).astype(BF16NP),
                "wk": np.ascontiguousarray(Wk[:, c0:c0 + GC]).astype(BF16NP),
                "wvp": wvp.astype(BF16NP),
                "wo": np.ascontiguousarray(Wo[c0:c0 + GC, :]).astype(BF16NP),
                "bq2": np.ascontiguousarray(bq2),
                "bk2": np.ascontiguousarray(bk2),
        })
    return in_maps


def kernel(query_input, kv_input, Wq, bq, Wkv, bkv, Wo, bo):
    global _nc_cache
    from concourse import bass_utils

    if _nc_cache is None:
        _nc_cache = _build_nc()
    nc = _nc_cache

    Wkv = np.asarray(Wkv, dtype=np.float32)
    Wo = np.asarray(Wo, dtype=np.float32)
    bo = np.asarray(bo, dtype=np.float32)
    bv = np.asarray(bkv, np.float32)[D:]

    in_maps = build_in_maps(dict(
        query_input=query_input, kv_input=kv_input, Wq=Wq, bq=bq,
        Wkv=Wkv, bkv=bkv, Wo=Wo))

    res = bass_utils.run_bass_kernel_spmd(nc, in_maps,
                                          core_ids=list(range(N_CORES)))

    # gather: sum the 4 head-group partials per batch; add biases the device
    # left out (bo, and bv which passes through Wo since softmax rows sum to 1)
    tail = bv @ Wo + bo
    out = np.empty((B, SQ, D), np.float32)
    for b in range(B):
        acc = res.results[b * G + 0]["out_p"].astype(np.float32)
        for g in range(1, G):
                acc = acc + res.results[b * G + g]["out_p"].astype(np.float32)
        out[b] = acc + tail[None, :]
    return out
